# revision 40
# baseline (speedup 1.0000x reference)
"""Trainium2 Bass kernel for nn_Cross_classifier (dense_cnn).

Pure data-parallel: batch 128 sharded across 8 NeuronCores (16 samples/core).
All parameters replicated. Self-contained: shapes hardcoded.

Math notes (exactly mirrors the reference):
  - f_z: Linear(1536->384) + LayerNorm + GELU on z = concat(z_r, z_i).
  - down_r/down_i: 3x3 SAME conv (768->384) + eval-BN + GELU, then center-crop
    16x16 -> 8x8.  Only the central 8x8 outputs are consumed, so we compute the
    conv only there, which needs just the central 10x10 input patch (100 of the
    256 tokens).  BN scale folds into the conv weights; conv bias + BN shift
    fold into a single per-channel bias applied inside the GELU activation.
  - xcorr: VALID correlation of an 8x8 kernel over an 8x8 map = per-sample dot
    product over (384 ch x 64 pos); then sigmoid(dot / c).

Implementation notes:
  - Matmuls run in bf16 (activations) x fp8e4m3 (conv weights) with fp32 PSUM
    accumulation.  The final sigmoid sits at ~sigmoid(10) where its derivative
    is ~5e-5, so low-precision products are far inside tolerance.
  - All contractions need the contraction dim on SBUF partitions, so z and the
    x patches are transposed on chip through the DMA xbar
    (dma_start_transpose, one batched op per input tile) on the SP HWDGE ring,
    keeping the PE free for matmuls.  fp32->bf16 casts feeding the xbar run on
    the otherwise idle GPSIMD engine so neither the DVE (LayerNorm) nor the
    rings gate them.
  - The 3x3 conv is 9 shifted-view matmuls (weights stationary, N=512 = 8
    samples x 64 positions) accumulated in PSUM.
  - x patches are stored per-sample in 112-wide columns (100 valid + 12
    zeroed) so the xbar 16-row alignment holds and tap views stay affine.
  - Pools use the queue allocator + double-buffered weight/XT slots so the
    second conv's input pipeline streams while the first conv computes.
"""

import numpy as np
import ml_dtypes

N_CORES = 8
B = 128
BPC = B // N_CORES      # samples per core: 16
T1 = 64                 # template tokens (8x8)
E = 768
E2 = 384
TWOE = 2 * E            # 1536
KCZ = TWOE // 128       # 12 contraction chunks for f_z
KC = E // 128           # 6 contraction chunks for conv
MC = E2 // 128          # 3 output-channel chunks
TOK = BPC * T1          # 1024 z tokens per core
NZT = TOK // 128        # 8 token tiles
NPATCH = 100            # 10x10 central input patch tokens per sample
PADP = 112              # NPATCH padded to a multiple of 16 for the xbar
GRP = BPC // 8          # sample groups of 8 (N=512 matmuls)
EPS = 1e-5

BF16 = ml_dtypes.bfloat16
FP8 = ml_dtypes.float8_e4m3

_PROG_CACHE: dict = {}


def _build_program(flags):
    """Build the per-core SPMD Bass/Tile program.

    flags = (has_fzb, has_lng, has_lnb): whether the f_z linear bias /
    LayerNorm gain / LayerNorm bias are non-trivial (they are structurally
    zero/one in this model; the general path is kept for robustness).
    """
    from contextlib import ExitStack
    import concourse.bass as bass
    import concourse.mybir as mybir
    import concourse.tile as tile
    from concourse import bacc

    has_fzb, has_lng, has_lnb = flags
    dt = mybir.dt
    f32, bf16, fp8 = dt.float32, dt.bfloat16, dt.float8e4
    AX = mybir.AxisListType
    OP = mybir.AluOpType
    AF = mybir.ActivationFunctionType

    nc = bacc.Bacc("TRN2", target_bir_lowering=False, debug=False,
                   num_devices=N_CORES)

    # ---- DRAM I/O ----
    z_d = nc.dram_tensor("z", [TOK, TWOE], f32, kind="ExternalInput")
    xr_d = nc.dram_tensor("xr", [BPC * NPATCH, E], f32, kind="ExternalInput")
    xi_d = nc.dram_tensor("xi", [BPC * NPATCH, E], f32, kind="ExternalInput")
    fzw_d = nc.dram_tensor("fzw", [KCZ, 128, E2], bf16, kind="ExternalInput")
    wr_d = nc.dram_tensor("wr", [KC, 128, 9, E2], fp8, kind="ExternalInput")
    wi_d = nc.dram_tensor("wi", [KC, 128, 9, E2], fp8, kind="ExternalInput")
    bshr_d = nc.dram_tensor("bshr", [MC, 128], f32, kind="ExternalInput")
    bshi_d = nc.dram_tensor("bshi", [MC, 128], f32, kind="ExternalInput")
    ones_d = nc.dram_tensor("ones", [128, 1], f32, kind="ExternalInput")
    c_d = nc.dram_tensor("c", [1, 1], f32, kind="ExternalInput")
    fzb_d = nc.dram_tensor("fzb", [1, E2], f32, kind="ExternalInput")
    lng_d = nc.dram_tensor("lng", [1, E2], f32, kind="ExternalInput")
    lnb_d = nc.dram_tensor("lnb", [1, E2], f32, kind="ExternalInput")
    s1_d = nc.dram_tensor("s1", [1, BPC], f32, kind="ExternalOutput")
    s2_d = nc.dram_tensor("s2", [1, BPC], f32, kind="ExternalOutput")

    def bcast_ap(handle):
        # Replicate a [1, N] DRAM row across 128 partitions (step-0 DMA).
        ap = handle.ap()
        return bass.AP(tensor=ap.tensor, offset=ap.offset,
                       ap=[[0, 128]] + [list(d) for d in ap.ap[1:]])

    with tile.TileContext(nc, pool_alloc_mode="queue") as tc, ExitStack() as ctx:
        const = ctx.enter_context(tc.tile_pool(name="const", bufs=1))

        fzw = const.tile([128, KCZ, E2], bf16)
        nc.sync.dma_start(out=fzw, in_=fzw_d.ap().rearrange("k p e -> p k e"))
        onesb = const.tile([128, 1], f32)
        nc.sync.dma_start(out=onesb, in_=ones_d.ap())
        ctile = const.tile([1, 1], f32)
        nc.sync.dma_start(out=ctile, in_=c_d.ap())
        invc = const.tile([1, 1], f32)
        nc.vector.reciprocal(invc, ctile)
        bshr = const.tile([128, MC], f32)
        nc.sync.dma_start(out=bshr, in_=bshr_d.ap().rearrange("m p -> p m"))
        bshi = const.tile([128, MC], f32)
        nc.sync.dma_start(out=bshi, in_=bshi_d.ap().rearrange("m p -> p m"))
        epst = const.tile([128, 1], f32)
        nc.vector.memset(epst, EPS)
        if has_fzb:
            fzb_bc = const.tile([128, E2], f32)
            nc.sync.dma_start(out=fzb_bc, in_=bcast_ap(fzb_d))
        if has_lng:
            lng_bc = const.tile([128, E2], f32)
            nc.sync.dma_start(out=lng_bc, in_=bcast_ap(lng_d))
        if has_lnb:
            lnb_bc = const.tile([128, E2], f32)
            nc.sync.dma_start(out=lnb_bc, in_=bcast_ap(lnb_d))

        # persistent across phases
        zgt_pool = ctx.enter_context(tc.tile_pool(name="zgt", bufs=1))
        ZGT = zgt_pool.tile([128, NZT, MC, 128], bf16)
        fin_pool = ctx.enter_context(tc.tile_pool(name="fin", bufs=1))
        dot_ps_pool = ctx.enter_context(
            tc.tile_pool(name="dotps", bufs=1, space="PSUM"))
        # conv pools (outer scope; two slots so conv-i streams during conv-r)
        wp = ctx.enter_context(tc.tile_pool(name="wsb", bufs=2))
        xtp = ctx.enter_context(tc.tile_pool(name="xt", bufs=2))
        xlp = ctx.enter_context(tc.tile_pool(name="xl", bufs=2))
        xbp = ctx.enter_context(tc.tile_pool(name="xb", bufs=2))
        xgp = ctx.enter_context(tc.tile_pool(name="xg", bufs=3))
        xcp = ctx.enter_context(tc.tile_pool(name="xc", bufs=4))
        cps = ctx.enter_context(tc.tile_pool(name="cps", bufs=2, space="PSUM"))


        def conv_inputs(tag, x_d, w_d, eng, cast_eng, xbars_last, gate=None):
            """Build the load/cast/transpose pipeline for one conv branch on
            the given HWDGE ring engine. Returns (XT0, XT1, wsb, thunks):
            thunks is a list of zero-arg emitters in ring order (weights,
            quad loads, quad transposes with one-quad lookahead) so the
            caller can interleave them with other ring traffic."""
            XT0 = xtp.tile([128, 8, KC, PADP], bf16, name=f"XT0{tag}",
                           tag="XT0", bufs=2)
            XT1 = xtp.tile([128, 8, KC, PADP], bf16, name=f"XT1{tag}",
                           tag="XT1", bufs=1)
            XTg = (XT0, XT1)
            wsb = wp.tile([128, KC, 9, E2], fp8, name=f"wsb{tag}", tag="wsb")
            # 4 samples per load: [100, 4, 768] (sample stride 100 rows in
            # DRAM maps to an affine AP); one cast, one memset, one batched
            # xbar transpose per quad
            xv = x_d.ap().rearrange("(s p) e -> p s e", p=NPATCH)

            def w_thunk():
                inst = eng.dma_start(out=wsb, in_=w_d.ap().rearrange(
                    "k p t e -> p k t e"))
                if gate is not None and gate() is not None:
                    tile.add_dep_helper(inst.ins, gate(), sync=True,
                                        reason="z pair 0 first on DMA")

            def load_thunk(a):
                xl = xlp.tile([NPATCH, 4, E], f32, name="xl", tag="xl")
                eng.dma_start(out=xl, in_=xv[:, 4 * a:4 * a + 4, :])
                xb = xbp.tile([PADP, 4, E], bf16)
                # zero the 12-row pad (aligned at 96; rows 96:100 are then
                # overwritten by the cast)
                nc.gpsimd.memset(xb[96:PADP, :, :], 0.0)
                cast_eng.tensor_copy(xb[0:NPATCH, :, :], xl)
                xbs[a] = xb

            def xbar_thunk(a):
                dst = XTg[a // 2][:, (a % 2) * 4:(a % 2) * 4 + 4, :, :]
                eng.dma_start_transpose(dst, xbs[a])

            xbs: list = [None] * (BPC // 4)
            thunks = [w_thunk, lambda: load_thunk(0), lambda: load_thunk(1)]
            if xbars_last:
                thunks += [lambda: load_thunk(2), lambda: load_thunk(3)]
                thunks += [lambda a=a: xbar_thunk(a) for a in range(4)]
            else:
                thunks += [lambda: xbar_thunk(0), lambda: load_thunk(2),
                           lambda: xbar_thunk(1), lambda: load_thunk(3),
                           lambda: xbar_thunk(2), lambda: xbar_thunk(3)]
            return XT0, XT1, wsb, thunks

        # ---------------- Z phase ----------------
        with tc.tile_pool(name="zload", bufs=2) as zlp, \
             tc.tile_pool(name="zcast", bufs=2) as zcp, \
             tc.tile_pool(name="zT", bufs=1) as ztp, \
             tc.tile_pool(name="zstat", bufs=4) as zsp, \
             tc.tile_pool(name="zg", bufs=4) as zgp, \
             tc.tile_pool(name="fzps", bufs=3, space="PSUM") as fzps:

            # z.T chunks: [e_local, zt, kc, tok_local]
            zT = ztp.tile([128, NZT, KCZ, 128], bf16)

            NPAIR = NZT // 2
            # token-tile-pair view of z: [pair, tok_local, j, e]
            zv = z_d.ap().rearrange("(a j p) e -> a p j e", j=2, p=128)
            zls: list = [None] * NPAIR

            def z_load(a):
                zls[a] = zlp.tile([128, 2, TWOE], f32, name="zl", tag="zl")
                nc.sync.dma_start(out=zls[a], in_=zv[a])

            first_z_xbar = [None]

            def z_xbar(a):
                zb = zcp.tile([128, 2, TWOE], bf16)
                nc.gpsimd.tensor_copy(zb, zls[a])
                inst = nc.sync.dma_start_transpose(
                    zT[:, 2 * a:2 * a + 2, :, :], zb)
                if first_z_xbar[0] is None:
                    first_z_xbar[0] = inst.ins

            # conv-r input thunks ride the ACT HWDGE ring; interleave their
            # emission between the z pairs so DMA-engine arbitration delivers
            # z pair 0 first (PE startup) and conv-r inputs just-in-time
            XTr0, XTr1, wsbr, r_thunks = conv_inputs(
                "r", xr_d, wr_d, nc.scalar, nc.gpsimd, xbars_last=False,
                gate=None)
            # z input pipeline on the SP ring: paired loads + batched xbar
            # transposes (two token tiles per op), with one-load lookahead;
            # casts on GPSIMD so neither DVE nor the ring gates a transpose
            per_pair = (0, 2, 2, 2)
            z_load(0)
            for a in range(NPAIR):
                if a + 1 < NPAIR:
                    z_load(a + 1)
                z_xbar(a)
                for _ in range(per_pair[a]):
                    if r_thunks:
                        r_thunks.pop(0)()
            for t in r_thunks:
                t()

            zg2all = zgp.tile([128, NZT, E2], bf16, tag="zg2all",
                              bufs=1)
            for zt in range(NZT):
                # f_z matmul: out[tok, ch] accumulated over 12 K-chunks
                ps = fzps.tile([128, E2], f32)
                for kc in range(KCZ):
                    nc.tensor.matmul(ps, lhsT=zT[:, zt, kc, :],
                                     rhs=fzw[:, kc, :],
                                     start=(kc == 0), stop=(kc == KCZ - 1))
                if has_fzb:
                    zf = zgp.tile([128, E2], f32, tag="zf32", bufs=2)
                    nc.vector.tensor_add(zf, ps, fzb_bc)
                    src = zf
                else:
                    src = ps
                # LayerNorm over the 384-ch free dim
                stats = zsp.tile([128, 6], f32, tag="stats")
                nc.vector.bn_stats(out=stats, in_=src)
                mv = zsp.tile([128, 2], f32, tag="mv")
                nc.vector.bn_aggr(out=mv, in_=stats)
                # rstd = 1/sqrt(var + eps)
                nc.scalar.activation(out=mv[:, 1:2], in_=mv[:, 1:2],
                                     func=AF.Sqrt, bias=epst, scale=1.0)
                nc.vector.reciprocal(mv[:, 1:2], mv[:, 1:2])
                zg = zgp.tile([128, E2], bf16, tag="zg", bufs=2)
                nc.vector.tensor_scalar(out=zg, in0=src,
                                        scalar1=mv[:, 0:1], scalar2=mv[:, 1:2],
                                        op0=OP.subtract, op1=OP.mult)
                if has_lng:
                    nc.vector.tensor_mul(zg, zg, lng_bc)
                if has_lnb:
                    nc.vector.tensor_add(zg, zg, lnb_bc)
                nc.scalar.activation(out=zg2all[:, zt, :], in_=zg,
                                     func=AF.Gelu)
            # single batched transpose of all gelu'd z to [ch, token]
            nc.sync.dma_start_transpose(ZGT[:, :, :, :], zg2all)

        def conv_compute(tag, XTg, wsb, bsh):
            D = fin_pool.tile([128, BPC], f32, tag=f"D{tag}")
            for g in range(GRP):
                zv = ZGT[:, 4 * g:4 * g + 4, :, :]  # [128, 4, MC, 128]
                for mc in range(MC):
                    pc = cps.tile([128, 512], f32)
                    n_mm = 9 * KC
                    i_mm = 0
                    for tap in range(9):
                        dy, dx = tap // 3, tap % 3
                        for kc in range(KC):
                            v = XTg[g][:, :, kc, :]
                            rhs = bass.AP(
                                tensor=v.tensor,
                                offset=v.offset + dy * 10 + dx,
                                ap=[list(v.ap[0]), list(v.ap[1]),
                                    [10, 8], [1, 8]])
                            nc.tensor.matmul(
                                pc,
                                lhsT=wsb[:, kc, tap, mc * 128:(mc + 1) * 128],
                                rhs=rhs,
                                start=(i_mm == 0), stop=(i_mm == n_mm - 1))
                            i_mm += 1
                    # fused BN-shift + GELU: gelu(conv + shift)
                    xg = xgp.tile([128, 512], bf16, tag="xg")
                    nc.scalar.activation(out=xg, in_=pc, func=AF.Gelu,
                                         bias=bsh[:, mc:mc + 1])
                    # xcorr partial: multiply by z_f, sum over positions
                    prod = xcp.tile([128, 4, 128], bf16, tag="prod")
                    nc.vector.tensor_mul(
                        prod, xg.rearrange("p (a b) -> p a b", a=4),
                        zv[:, :, mc, :])
                    red = xcp.tile([128, 8], f32, tag="red")
                    nc.vector.tensor_reduce(
                        out=red,
                        in_=prod.rearrange("p a b -> p (a b)").rearrange(
                            "p (s q) -> p s q", q=T1),
                        axis=AX.X, op=OP.add)
                    dsl = D[:, g * 8:(g + 1) * 8]
                    if mc == 0:
                        nc.vector.tensor_copy(dsl, red)
                    else:
                        nc.vector.tensor_add(dsl, dsl, red)
            # cross-partition sum via ones-matmul
            dot = dot_ps_pool.tile([1, BPC], f32, tag=f"dot{tag}")
            nc.tensor.matmul(dot, lhsT=onesb, rhs=D, start=True, stop=True)
            return dot

        # conv-i inputs stream on the SP ring (idle after the z phase) while
        # conv-r computes; its group-1 transposes wait for conv-r's reads of
        # the shared XT1 slot, so they go last on the ring
        XTi0, XTi1, wsbi, i_thunks = conv_inputs(
            "i", xi_d, wi_d, nc.sync, nc.gpsimd, xbars_last=False)
        for t in i_thunks:
            t()
        dot_r = conv_compute("r", (XTr0, XTr1), wsbr, bshr)
        dot_i = conv_compute("i", (XTi0, XTi1), wsbi, bshi)

        # sigmoid(dot / c) for both branches last (single act-table switch)
        sg_r = fin_pool.tile([1, BPC], f32, tag="sgr")
        nc.scalar.activation(out=sg_r, in_=dot_r, func=AF.Sigmoid,
                             scale=invc[0:1, 0:1])
        nc.sync.dma_start(out=s1_d.ap(), in_=sg_r)
        sg_i = fin_pool.tile([1, BPC], f32, tag="sgi")
        nc.scalar.activation(out=sg_i, in_=dot_i, func=AF.Sigmoid,
                             scale=invc[0:1, 0:1])
        nc.sync.dma_start(out=s2_d.ap(), in_=sg_i)

    nc.finalize()
    return nc


def get_program(flags=(False, False, False)):
    if flags not in _PROG_CACHE:
        _PROG_CACHE[flags] = _build_program(flags)
    return _PROG_CACHE[flags]


def prep_inputs(z_r, z_i, x_r, x_i, fz_w, fz_b, ln_g, ln_b,
                wr, br, bnr_g, bnr_b, bnr_m, bnr_v,
                wi, bi, bni_g, bni_b, bni_m, bni_v, c):
    """Host-side sharding + offline weight packing. Returns (flags, in_maps)."""
    z_r = np.asarray(z_r, np.float32)
    z_i = np.asarray(z_i, np.float32)
    x_r = np.asarray(x_r, np.float32)
    x_i = np.asarray(x_i, np.float32)

    # template branch: z = concat(z_r, z_i) per sample -> [B*T1, 1536]
    z = np.concatenate([z_r, z_i], axis=2)

    # search branch: central 10x10 patch of each 16x16 token grid
    def patches(x):
        xv = x.reshape(B, 16, 16, E)[:, 3:13, 3:13, :]
        return np.ascontiguousarray(xv).reshape(B, NPATCH, E)
    xpr = patches(x_r)
    xpi = patches(x_i)

    # f_z weight: [E2, 2E] -> transposed chunks [KCZ, 128, E2]
    fzw_pack = np.ascontiguousarray(
        np.asarray(fz_w, np.float32).T.reshape(KCZ, 128, E2)).astype(BF16)

    # conv weights with BN scale folded; bias+BN shift folded to one vector
    def fold(w, b, g, beta, m, v):
        w = np.asarray(w, np.float32)
        scale = np.asarray(g, np.float32) / np.sqrt(np.asarray(v, np.float32) + EPS)
        shift = (np.asarray(b, np.float32) - np.asarray(m, np.float32)) * scale \
            + np.asarray(beta, np.float32)
        wt = (w * scale[:, None, None, None]).transpose(1, 2, 3, 0)  # [ci,3,3,co]
        wt = np.ascontiguousarray(wt.reshape(KC, 128, 9, E2)).astype(FP8)
        return wt, shift.reshape(MC, 128).astype(np.float32)
    wr_pack, bshr = fold(wr, br, bnr_g, bnr_b, bnr_m, bnr_v)
    wi_pack, bshi = fold(wi, bi, bni_g, bni_b, bni_m, bni_v)

    fzb = np.asarray(fz_b, np.float32).reshape(1, E2)
    lng = np.asarray(ln_g, np.float32).reshape(1, E2)
    lnb = np.asarray(ln_b, np.float32).reshape(1, E2)
    flags = (bool(np.any(fzb)), not bool(np.all(lng == 1.0)), bool(np.any(lnb)))

    shared = {
        "fzw": fzw_pack, "wr": wr_pack, "wi": wi_pack,
        "bshr": bshr, "bshi": bshi,
        "ones": np.ones((128, 1), np.float32),
        "c": np.asarray(c, np.float32).reshape(1, 1),
        "fzb": fzb, "lng": lng, "lnb": lnb,
    }
    in_maps = []
    for core in range(N_CORES):
        sl = slice(core * BPC, (core + 1) * BPC)
        m = dict(shared)
        m["z"] = np.ascontiguousarray(z[sl]).reshape(TOK, TWOE)
        m["xr"] = np.ascontiguousarray(xpr[sl]).reshape(BPC * NPATCH, E)
        m["xi"] = np.ascontiguousarray(xpi[sl]).reshape(BPC * NPATCH, E)
        in_maps.append(m)
    return flags, in_maps


def kernel(**inputs):
    from concourse.bass_utils import run_bass_kernel_spmd

    flags, in_maps = prep_inputs(**inputs)
    nc = get_program(flags)
    res = run_bass_kernel_spmd(nc, in_maps, core_ids=list(range(N_CORES)))
    s1 = np.concatenate([np.asarray(res.results[i]["s1"]).reshape(-1)
                         for i in range(N_CORES)])
    s2 = np.concatenate([np.asarray(res.results[i]["s2"]).reshape(-1)
                         for i in range(N_CORES)])
    return (s1.reshape(B, 1, 1, 1).astype(np.float32),
            s2.reshape(B, 1, 1, 1).astype(np.float32))
